# revision 21
# baseline (speedup 1.0000x reference)
"""Embedding-lookup dot product kernel for 8 TRN2 NeuronCores.

out[i] = dot(user_matrix[location[i,0], :], goods_matrix[:, location[i,1]])

Strategy: data-parallel over the 16384-pair batch (2048 pairs/core); both
factor matrices are replicated into each core's DRAM (HBM is plentiful, no
collectives needed). goods_matrix is transposed on the host once so a column
gather becomes a contiguous 512B row gather. On-device, each core runs 32
indirect DMA gathers (128 rows x 512B each, one row per SBUF partition;
hardware indirect DMA consumes exactly one offset per partition), with
elementwise multiply + per-row reduction on the vector engine overlapped
chunk-by-chunk under the gather stream. Raw Bass Block (no TileContext) to
minimize fixed overhead. The wall is the Pool engine's serial SWDGE
descriptor generation (~1.1us per 128-row indirect DMA).
"""

from contextlib import ExitStack

import numpy as np

import concourse.bacc as bacc
import concourse.mybir as mybir
from concourse import bass
from concourse.bass_utils import run_bass_kernel_spmd

N_CORES = 8
USER_NUM = 500000
GOODS_NUM = 500000
K = 128               # embedding dim
BATCH = 16384
P = 128               # SBUF partitions
PER_CORE = BATCH // N_CORES          # 2048 pairs per core
J = PER_CORE // P                    # 16 pairs per partition

_CACHE = {}


def build_nc():
    """Build + compile the per-core Bass graph (identical on all 8 cores)."""
    f32 = mybir.dt.float32
    i32 = mybir.dt.int32

    nc = bacc.Bacc(
        "TRN2",
        target_bir_lowering=False,
        debug=False,
        num_devices=N_CORES,
        enable_partition_id=False,
        monotonic_sem_count=0,
    )
    user = nc.dram_tensor("user", [USER_NUM, K], f32, kind="ExternalInput")
    goodsT = nc.dram_tensor("goodsT", [GOODS_NUM, K], f32, kind="ExternalInput")
    # user and goods indices side by side: one 16KB load instead of two 8KB
    loc = nc.dram_tensor("loc", [P, 2 * J], i32, kind="ExternalInput")
    out = nc.dram_tensor("out", [P, J], f32, kind="ExternalOutput")

    with (
        nc.Block() as block,
        nc.sbuf_tensor("idx", [P, 2 * J], i32) as idx,
        nc.sbuf_tensor("ut", [P, J, K], f32) as ut,
        nc.sbuf_tensor("gt", [P, J, K], f32) as gt,
        nc.sbuf_tensor("res", [P, J], f32) as res,
        nc.semaphore("io") as io,
        nc.semaphore("vsem") as vsem,
        nc.semaphore("msem") as msem,
        nc.semaphore("udummy") as udummy,
        ExitStack() as stack,
    ):
        # one dedicated sem per chunk's gather pair: a threshold of 32 then
        # means exactly "both of this chunk's DMAs fully landed" (a shared
        # cumulative sem would be racy — per-engine incs from later pipelined
        # DMAs on the same queue interleave with earlier ones)
        gsems = [stack.enter_context(nc.semaphore(f"g{j}")) for j in range(J)]  # noqa: ANT232

        @block.sync
        def _(sync):
            sync.dma_start(out=idx[:], in_=loc[:]).then_inc(io, 16)
            # store results once the vector engine finishes all J chunks
            sync.wait_ge(vsem, J)
            sync.dma_start(out=out[:], in_=res[:]).then_inc(io, 16)
            sync.wait_ge(io, 32)

        @block.gpsimd
        def _(gpsimd):
            gpsimd.wait_ge(io, 16)
            for j in range(J):
                # u-gather incs only a dummy sem nobody waits on: the SDMA
                # rings drain FIFO per engine, so all 16 engine-incs from the
                # g-gather (queued after it on the same ring set) imply the
                # u-gather's data landed too
                gpsimd.indirect_dma_start(
                    out=ut[:, j],
                    out_offset=None,
                    in_=user[:],
                    in_offset=bass.IndirectOffsetOnAxis(
                        ap=idx[:, j:j + 1], axis=0
                    ),
                ).then_inc(udummy, 16)
                gpsimd.indirect_dma_start(
                    out=gt[:, j],
                    out_offset=None,
                    in_=goodsT[:],
                    in_offset=bass.IndirectOffsetOnAxis(
                        ap=idx[:, J + j:J + j + 1], axis=0
                    ),
                ).then_inc(gsems[j], 16)

        @block.vector
        def _(vector):
            for j in range(J):
                vector.wait_ge(gsems[j], 16)
                vector.tensor_mul(out=ut[:, j], in0=ut[:, j], in1=gt[:, j]).then_inc(
                    msem, 1
                )
                # same-engine RAW still needs a sem: DVE writes drain async
                vector.wait_ge(msem, j + 1)
                vector.tensor_reduce(
                    out=res[:, j:j + 1],
                    in_=ut[:, j],
                    axis=mybir.AxisListType.X,
                    op=mybir.AluOpType.add,
                ).then_inc(vsem, 1)

    nc.compile()
    return nc


def _get_nc():
    if "nc" not in _CACHE:
        _CACHE["nc"] = build_nc()
    return _CACHE["nc"]


def make_in_maps(user_matrix, goods_matrix, location):
    """Host-side sharding: transpose goods, split batch 8 ways, cast idx."""
    user = np.ascontiguousarray(np.asarray(user_matrix), dtype=np.float32)
    goodsT = np.ascontiguousarray(np.asarray(goods_matrix).T).astype(
        np.float32, copy=False
    )
    loc = np.asarray(location).astype(np.int32)
    # pair b of core c sits at partition (b % 2048) // 16, slot b % 16;
    # per-core idx tile is [P, 2J]: user indices in cols [0,J), goods in [J,2J)
    lu = loc[:, 0].reshape(N_CORES, P, J)
    lg = loc[:, 1].reshape(N_CORES, P, J)
    merged = np.concatenate([lu, lg], axis=2)
    return [
        {"user": user, "goodsT": goodsT, "loc": merged[i]}
        for i in range(N_CORES)
    ]


def run(in_maps, trace=False, **kwargs):
    nc = _get_nc()
    return run_bass_kernel_spmd(
        nc, in_maps, core_ids=list(range(N_CORES)), trace=trace, **kwargs
    )


def kernel(user_matrix, goods_matrix, location):
    in_maps = make_in_maps(user_matrix, goods_matrix, location)
    res = run(in_maps)
    out = np.concatenate(
        [res.results[i]["out"].reshape(-1) for i in range(N_CORES)]
    )
    return out.reshape(BATCH, 1).astype(np.float32)


# revision 23
# speedup vs baseline: 1.1802x; 1.1802x over previous
"""Embedding-lookup dot product kernel for 8 TRN2 NeuronCores.

out[i] = dot(user_matrix[location[i,0], :], goods_matrix[:, location[i,1]])

Strategy: data-parallel over the 16384-pair batch (2048 pairs/core); both
factor matrices are replicated into each core's DRAM (HBM is plentiful, no
collectives needed). goods_matrix is transposed on the host once so a column
gather becomes a contiguous 512B row gather. On-device, each core runs 32
indirect DMA gathers (128 rows x 512B each, one row per SBUF partition;
hardware indirect DMA consumes exactly one offset per partition), with
elementwise multiply + per-row reduction on the vector engine overlapped
chunk-by-chunk under the gather stream. Raw Bass Block (no TileContext) to
minimize fixed overhead. The wall is the Pool engine's serial SWDGE
descriptor generation (~1.1us per 128-row indirect DMA).
"""

from contextlib import ExitStack

import numpy as np

import concourse.bacc as bacc
import concourse.mybir as mybir
from concourse import bass
from concourse.bass_utils import run_bass_kernel_spmd

N_CORES = 8
USER_NUM = 500000
GOODS_NUM = 500000
K = 128               # embedding dim
BATCH = 16384
P = 128               # SBUF partitions
PER_CORE = BATCH // N_CORES          # 2048 pairs per core
J = PER_CORE // P                    # 16 pairs per partition

_CACHE = {}


def build_nc():
    """Build + compile the per-core Bass graph (identical on all 8 cores)."""
    f32 = mybir.dt.float32
    i32 = mybir.dt.int32

    nc = bacc.Bacc(
        "TRN2",
        target_bir_lowering=False,
        debug=False,
        num_devices=N_CORES,
        enable_partition_id=False,
        monotonic_sem_count=0,
        num_swdge_queues=2,
    )
    user = nc.dram_tensor("user", [USER_NUM, K], f32, kind="ExternalInput")
    goodsT = nc.dram_tensor("goodsT", [GOODS_NUM, K], f32, kind="ExternalInput")
    # user and goods indices side by side: one 16KB load instead of two 8KB
    loc = nc.dram_tensor("loc", [P, 2 * J], i32, kind="ExternalInput")
    out = nc.dram_tensor("out", [P, J], f32, kind="ExternalOutput")

    with (
        nc.Block() as block,
        nc.sbuf_tensor("idx", [P, 2 * J], i32) as idx,
        nc.sbuf_tensor("ut", [P, J, K], f32) as ut,
        nc.sbuf_tensor("gt", [P, J, K], f32) as gt,
        nc.sbuf_tensor("res", [P, J], f32) as res,
        nc.semaphore("io") as io,
        nc.semaphore("vsem") as vsem,
        nc.semaphore("msem") as msem,
        nc.semaphore("udummy") as udummy,
        ExitStack() as stack,
    ):
        # one dedicated sem per chunk's gather pair: a threshold of 32 then
        # means exactly "both of this chunk's DMAs fully landed" (a shared
        # cumulative sem would be racy — per-engine incs from later pipelined
        # DMAs on the same queue interleave with earlier ones)
        gsems = [stack.enter_context(nc.semaphore(f"g{j}")) for j in range(J)]  # noqa: ANT232

        @block.sync
        def _(sync):
            sync.dma_start(out=idx[:], in_=loc[:]).then_inc(io, 16)
            # store results once the vector engine finishes all J chunks
            sync.wait_ge(vsem, J)
            sync.dma_start(out=out[:], in_=res[:]).then_inc(io, 16)
            sync.wait_ge(io, 32)

        @block.gpsimd
        def _(gpsimd):
            gpsimd.wait_ge(io, 16)
            for j in range(J):
                # u-gather incs only a dummy sem nobody waits on: the SDMA
                # rings drain FIFO per engine, so all 16 engine-incs from the
                # g-gather (queued after it on the same ring set) imply the
                # u-gather's data landed too
                # alternate SWDGE queue contexts per chunk so ring metadata
                # bookkeeping of consecutive instructions overlaps; both
                # gathers of a chunk stay on one queue to keep the FIFO
                # argument above valid
                q = f"qPoolDynamic{j % 2 or ''}"
                gpsimd.indirect_dma_start(
                    out=ut[:, j],
                    out_offset=None,
                    in_=user[:],
                    in_offset=bass.IndirectOffsetOnAxis(
                        ap=idx[:, j:j + 1], axis=0
                    ),
                ).then_inc(udummy, 16).ins.queue = q
                gpsimd.indirect_dma_start(
                    out=gt[:, j],
                    out_offset=None,
                    in_=goodsT[:],
                    in_offset=bass.IndirectOffsetOnAxis(
                        ap=idx[:, J + j:J + j + 1], axis=0
                    ),
                ).then_inc(gsems[j], 16).ins.queue = q

        @block.vector
        def _(vector):
            for j in range(J):
                vector.wait_ge(gsems[j], 16)
                vector.tensor_mul(out=ut[:, j], in0=ut[:, j], in1=gt[:, j]).then_inc(
                    msem, 1
                )
                # same-engine RAW still needs a sem: DVE writes drain async
                vector.wait_ge(msem, j + 1)
                vector.tensor_reduce(
                    out=res[:, j:j + 1],
                    in_=ut[:, j],
                    axis=mybir.AxisListType.X,
                    op=mybir.AluOpType.add,
                ).then_inc(vsem, 1)

    nc.compile()
    return nc


def _get_nc():
    if "nc" not in _CACHE:
        _CACHE["nc"] = build_nc()
    return _CACHE["nc"]


def make_in_maps(user_matrix, goods_matrix, location):
    """Host-side sharding: transpose goods, split batch 8 ways, cast idx."""
    user = np.ascontiguousarray(np.asarray(user_matrix), dtype=np.float32)
    goodsT = np.ascontiguousarray(np.asarray(goods_matrix).T).astype(
        np.float32, copy=False
    )
    loc = np.asarray(location).astype(np.int32)
    # pair b of core c sits at partition (b % 2048) // 16, slot b % 16;
    # per-core idx tile is [P, 2J]: user indices in cols [0,J), goods in [J,2J)
    lu = loc[:, 0].reshape(N_CORES, P, J)
    lg = loc[:, 1].reshape(N_CORES, P, J)
    merged = np.concatenate([lu, lg], axis=2)
    return [
        {"user": user, "goodsT": goodsT, "loc": merged[i]}
        for i in range(N_CORES)
    ]


def run(in_maps, trace=False, **kwargs):
    nc = _get_nc()
    return run_bass_kernel_spmd(
        nc, in_maps, core_ids=list(range(N_CORES)), trace=trace, **kwargs
    )


def kernel(user_matrix, goods_matrix, location):
    in_maps = make_in_maps(user_matrix, goods_matrix, location)
    res = run(in_maps)
    out = np.concatenate(
        [res.results[i]["out"].reshape(-1) for i in range(N_CORES)]
    )
    return out.reshape(BATCH, 1).astype(np.float32)
